# revision 1
# baseline (speedup 1.0000x reference)
"""GQA cross-attention block on 8 trn2 NeuronCores.

Sharding: tensor-parallel over heads. Core c owns KV group g=c (64 dims of
K/V) and its 4 query heads (256 q channels). Each core computes its heads'
attention plus its slice of the o-projection (rows c*256:(c+1)*256 of Wo),
producing a full-shape partial output; the host sums the 8 partials and
adds bo. No device collectives needed.

Device layouts (host prepares):
  xT, encT: [B, HIDDEN, S] bf16 (activations transposed so hidden lands on
  the PE contraction/partition dim), per-core weight slices in bf16,
  biases as [P, 1] fp32 columns for ACT's per-partition bias.

On-chip dataflow per (batch):
  qT [256c, S] = Wq_c^T @ xT   (PE, accum over 16 hidden chunks)
  kT [64, S], vT [64, S] from encT; vT transposed to v_aug [128k, 65]
  chunks with a ones column (row 64 of the AV matmul then yields the
  softmax denominator Z for free).
  scores^T [128k, 512q] = kT-chunk.T @ qT-head-slice (PE)
  E = exp(0.125 * scores) (ACT, PSUM->SBUF bf16)
  av_aug [65, 512q] += v_aug.T @ E (PE, accum over 16 k chunks)
  oT [64, 512q] = av * broadcast(1/Z)  (recip on DVE, broadcast via a
  K=1 PE matmul with a ones row, multiply on DVE)
  out_partial [128s, 512h] = oT.T @ Wo_c (PE) -> bf16 -> DRAM
"""

import numpy as np
import ml_dtypes

import concourse.bass as bass
from concourse import bacc
import concourse.mybir as mybir
import concourse.tile as tile
from concourse.bass_utils import run_bass_kernel_spmd
from concourse.masks import make_identity

BF16 = ml_dtypes.bfloat16
F32 = mybir.dt.float32
BF = mybir.dt.bfloat16

B = 2
S = 2048
HID = 2048
D = 64          # head dim
RQ = 4          # query heads per core (per kv group)
CH = RQ * D     # 256 q channels per core
NCORES = 8
NH = HID // 128  # 16 hidden chunks
NST = S // 512   # 4 s-tiles of 512
NKC = S // 128   # 16 key chunks of 128
SCALE = 1.0 / np.sqrt(D)


def _build_nc() -> bass.Bass:
    nc = bacc.Bacc()

    xT = nc.dram_tensor("xT", [B, HID, S], BF, kind="ExternalInput")
    encT = nc.dram_tensor("encT", [B, HID, S], BF, kind="ExternalInput")
    wq = nc.dram_tensor("wq", [HID, CH], BF, kind="ExternalInput")
    wk = nc.dram_tensor("wk", [HID, D], BF, kind="ExternalInput")
    wv = nc.dram_tensor("wv", [HID, D], BF, kind="ExternalInput")
    wo = nc.dram_tensor("wo", [CH, HID], BF, kind="ExternalInput")
    bq = nc.dram_tensor("bq", [CH, 1], F32, kind="ExternalInput")
    bk = nc.dram_tensor("bk", [D, 1], F32, kind="ExternalInput")
    bv = nc.dram_tensor("bv", [D, 1], F32, kind="ExternalInput")
    out = nc.dram_tensor("out", [B, S, HID], BF, kind="ExternalOutput")

    with tile.TileContext(nc) as tc:
        with (
            tc.tile_pool(name="wpool", bufs=1) as wpool,
            tc.tile_pool(name="xs", bufs=6) as xs_pool,
            tc.tile_pool(name="es", bufs=6) as es_pool,
            tc.tile_pool(name="acts", bufs=2) as acts,
            tc.tile_pool(name="vaug", bufs=2 * NKC) as vaug_pool,
            tc.tile_pool(name="epool", bufs=8) as epool,
            tc.tile_pool(name="small", bufs=4) as small,
            tc.tile_pool(name="osb", bufs=4) as osb_pool,
            tc.tile_pool(name="psum", bufs=2, space="PSUM") as ps,
        ):
            # ---- resident constants / weights ----
            wq_t = []
            wk_t = []
            wv_t = []
            for h in range(NH):
                wqh = wpool.tile([128, CH], BF, name=f"wq{h}")
                nc.sync.dma_start(out=wqh[:], in_=wq[h * 128:(h + 1) * 128, :])
                wq_t.append(wqh)
                wkh = wpool.tile([128, D], BF, name=f"wk{h}")
                nc.sync.dma_start(out=wkh[:], in_=wk[h * 128:(h + 1) * 128, :])
                wk_t.append(wkh)
                wvh = wpool.tile([128, D], BF, name=f"wv{h}")
                nc.sync.dma_start(out=wvh[:], in_=wv[h * 128:(h + 1) * 128, :])
                wv_t.append(wvh)
            wo_t = []
            for cchunk in range(2):
                woc = wpool.tile([128, HID], BF, name=f"wo{cchunk}")
                nc.sync.dma_start(out=woc[:], in_=wo[cchunk * 128:(cchunk + 1) * 128, :])
                wo_t.append(woc)
            bq_t = []
            for cchunk in range(2):
                bqc = wpool.tile([128, 1], F32, name=f"bq{cchunk}")
                nc.sync.dma_start(out=bqc[:], in_=bq[cchunk * 128:(cchunk + 1) * 128, :])
                bq_t.append(bqc)
            bk_t = wpool.tile([D, 1], F32, name="bk_t")
            nc.sync.dma_start(out=bk_t[:], in_=bk[:, :])
            bv_t = wpool.tile([D, 1], F32, name="bv_t")
            nc.sync.dma_start(out=bv_t[:], in_=bv[:, :])

            ident = wpool.tile([128, 128], BF, name="ident")
            make_identity(nc, ident[:])
            ones1 = wpool.tile([1, D], F32, name="ones1")
            nc.gpsimd.memset(ones1[:], 1.0)

            ID = mybir.ActivationFunctionType.Identity
            EXP = mybir.ActivationFunctionType.Exp

            for b in range(B):
                # ---- phase A: projections ----
                qT_h = [
                    acts.tile([D, S], BF, tag=f"q{r}", name=f"q{r}_{b}")
                    for r in range(RQ)
                ]
                kT = acts.tile([D, S], BF, tag="kT", name=f"kT{b}")
                vT = acts.tile([D, S], BF, tag="vT", name=f"vT{b}")

                for st in range(NST):
                    ssl = slice(st * 512, (st + 1) * 512)
                    qps_lo = ps.tile([128, 512], F32, tag="pproj", name=f"qpl{b}{st}")
                    qps_hi = ps.tile([128, 512], F32, tag="pproj", name=f"qph{b}{st}")
                    for h in range(NH):
                        xt = xs_pool.tile([128, 512], BF, tag="xs", name=f"xs{b}{st}{h}")
                        nc.gpsimd.dma_start(
                            out=xt[:], in_=xT[b, h * 128:(h + 1) * 128, ssl])
                        nc.tensor.matmul(
                            qps_lo[:], wq_t[h][:, 0:128], xt[:],
                            start=(h == 0), stop=(h == NH - 1))
                        nc.tensor.matmul(
                            qps_hi[:], wq_t[h][:, 128:256], xt[:],
                            start=(h == 0), stop=(h == NH - 1))
                    for r in range(RQ):
                        src = qps_lo if r < 2 else qps_hi
                        row = (r % 2) * D
                        nc.scalar.activation(
                            qT_h[r][:, ssl], src[row:row + D, :], ID,
                            bias=bq_t[r // 2][row:row + D, :])

                for st in range(NST):
                    ssl = slice(st * 512, (st + 1) * 512)
                    kps = ps.tile([D, 512], F32, tag="pproj", name=f"kps{b}{st}")
                    vps = ps.tile([D, 512], F32, tag="pproj", name=f"vps{b}{st}")
                    for h in range(NH):
                        et = es_pool.tile([128, 512], BF, tag="es", name=f"es{b}{st}{h}")
                        nc.gpsimd.dma_start(
                            out=et[:], in_=encT[b, h * 128:(h + 1) * 128, ssl])
                        nc.tensor.matmul(
                            kps[:], wk_t[h][:], et[:],
                            start=(h == 0), stop=(h == NH - 1))
                        nc.tensor.matmul(
                            vps[:], wv_t[h][:], et[:],
                            start=(h == 0), stop=(h == NH - 1))
                    nc.scalar.activation(kT[:, ssl], kps[:], ID, bias=bk_t[:])
                    nc.scalar.activation(vT[:, ssl], vps[:], ID, bias=bv_t[:])

                # v_aug chunks: [128 kpos, 65] with ones in col 64
                v_aug = []
                for kc in range(NKC):
                    vtp = ps.tile([128, D], BF, tag="ps", name=f"vtp{b}{kc}")
                    nc.tensor.transpose(
                        vtp[:], vT[:, kc * 128:(kc + 1) * 128], ident[0:D, 0:D])
                    va = vaug_pool.tile([128, D + 1], BF, tag="vaug", name=f"va{b}{kc}")
                    nc.gpsimd.memset(va[:, D:D + 1], 1.0)
                    nc.vector.tensor_copy(va[:, 0:D], vtp[:])
                    v_aug.append(va)

                # ---- attention + oT ----
                oT_lo = acts.tile([128, S], BF, tag="olo", name=f"olo{b}")
                oT_hi = acts.tile([128, S], BF, tag="ohi", name=f"ohi{b}")
                for r in range(RQ):
                    odst = oT_lo if r < 2 else oT_hi
                    row = (r % 2) * D
                    avs = [
                        ps.tile([D + 1, 512], F32, tag="pav", bufs=4,
                                name=f"av{b}{r}{qc}")
                        for qc in range(NST)
                    ]
                    # kc-outer, qc pairs inner: stationary (kT chunk /
                    # v_aug chunk) reused across consecutive matmuls, and
                    # all four av accumulators stay hot in PSUM.
                    for kc in range(NKC):
                        ksl = slice(kc * 128, (kc + 1) * 128)
                        for pair in range(NST // 2):
                            scs = []
                            for qc in (2 * pair, 2 * pair + 1):
                                qsl = slice(qc * 512, (qc + 1) * 512)
                                sct = ps.tile([128, 512], F32, tag="ps",
                                              name=f"sc{b}{r}{qc}{kc}")
                                nc.tensor.matmul(
                                    sct[:], kT[:, ksl], qT_h[r][:, qsl],
                                    start=True, stop=True)
                                e_t = epool.tile([128, 512], BF, tag="e",
                                                 name=f"e{b}{r}{qc}{kc}")
                                nc.scalar.activation(
                                    e_t[:], sct[:], EXP, scale=float(SCALE))
                                scs.append(e_t)
                            for j, qc in enumerate((2 * pair, 2 * pair + 1)):
                                nc.tensor.matmul(
                                    avs[qc][:], v_aug[kc][:], scs[j][:],
                                    start=(kc == 0), stop=(kc == NKC - 1))
                    for qc in range(NST):
                        qsl = slice(qc * 512, (qc + 1) * 512)
                        av = avs[qc]
                        rt = small.tile([1, 512], F32, tag="rt", name=f"rt{b}{r}{qc}")
                        nc.vector.reciprocal(rt[:], av[D:D + 1, :])
                        bc = ps.tile([D, 512], F32, tag="ps", name=f"bc{b}{r}{qc}")
                        nc.tensor.matmul(bc[:], ones1[:], rt[:], start=True, stop=True)
                        bcs = small.tile([D, 512], F32, tag="bcs", name=f"bcs{b}{r}{qc}")
                        nc.vector.tensor_copy(bcs[:], bc[:])
                        nc.vector.tensor_mul(odst[row:row + D, qsl], av[0:D, :], bcs[:])

                # ---- o-projection (partial over this core's 256 channels) ----
                for sc16 in range(S // 128):
                    s128 = slice(sc16 * 128, (sc16 + 1) * 128)
                    for hc in range(HID // 512):
                        hsl = slice(hc * 512, (hc + 1) * 512)
                        ops = ps.tile([128, 512], F32, tag="pproj", name=f"op{b}{sc16}{hc}")
                        nc.tensor.matmul(
                            ops[:], oT_lo[:, s128], wo_t[0][:, hsl],
                            start=True, stop=False)
                        nc.tensor.matmul(
                            ops[:], oT_hi[:, s128], wo_t[1][:, hsl],
                            start=False, stop=True)
                        osb = osb_pool.tile([128, 512], BF, tag="osb", name=f"ob{b}{sc16}{hc}")
                        nc.vector.tensor_copy(osb[:], ops[:])
                        nc.sync.dma_start(out=out[b, s128, hsl], in_=osb[:])

    if not nc.is_finalized():
        nc.finalize()
    return nc


_NC = None
_RUN_KWARGS = {}
_LAST_RESULT = None


def _get_nc():
    global _NC
    if _NC is None:
        _NC = _build_nc()
    return _NC


def kernel(x, encoder_output, Wq, bq, Wk, bk, Wv, bv, Wo, bo):
    nc = _get_nc()
    xT = np.ascontiguousarray(np.asarray(x, np.float32).transpose(0, 2, 1)).astype(BF16)
    encT = np.ascontiguousarray(
        np.asarray(encoder_output, np.float32).transpose(0, 2, 1)).astype(BF16)
    Wq = np.asarray(Wq, np.float32)
    Wk = np.asarray(Wk, np.float32)
    Wv = np.asarray(Wv, np.float32)
    Wo = np.asarray(Wo, np.float32)
    in_maps = []
    for c in range(NCORES):
        csl = slice(c * CH, (c + 1) * CH)
        gsl = slice(c * D, (c + 1) * D)
        in_maps.append({
            "xT": xT,
            "encT": encT,
            "wq": np.ascontiguousarray(Wq[:, csl]).astype(BF16),
            "wk": np.ascontiguousarray(Wk[:, gsl]).astype(BF16),
            "wv": np.ascontiguousarray(Wv[:, gsl]).astype(BF16),
            "wo": np.ascontiguousarray(Wo[csl, :]).astype(BF16),
            "bq": np.ascontiguousarray(
                np.asarray(bq, np.float32)[csl].reshape(CH, 1)),
            "bk": np.ascontiguousarray(
                np.asarray(bk, np.float32)[gsl].reshape(D, 1)),
            "bv": np.ascontiguousarray(
                np.asarray(bv, np.float32)[gsl].reshape(D, 1)),
        })
    res = run_bass_kernel_spmd(nc, in_maps, list(range(NCORES)), **_RUN_KWARGS)
    global _LAST_RESULT
    _LAST_RESULT = res
    total = np.zeros((B, S, HID), np.float32)
    for c in range(NCORES):
        total += res.results[c]["out"].astype(np.float32)
    return total + np.asarray(bo, np.float32)



# revision 5
# speedup vs baseline: 1.9788x; 1.9788x over previous
"""GQA cross-attention block on 8 trn2 NeuronCores (v2).

Sharding: tensor-parallel over heads. Core c owns KV group g=c (64 dims of
K/V) and its 4 query heads (256 q channels). Each core computes its heads'
attention plus its slice of the o-projection (rows c*256:(c+1)*256 of Wo),
producing a full-shape partial output; the host sums the 8 partials and
adds bo. No device collectives needed.

v2 schedule (vs v1): K|V projection fused into one M=128 matmul chain;
scores row-tiled via tile_position so two heads' K=64 matmuls share the PE
array; exp runs on [128,1024] two-bank PSUM spans (half the ACT instruction
overhead); softmax normalization uses reciprocal_approx_fast + gpsimd
partition_broadcast instead of the 1-partition DVE reciprocal + ones-matmul
chain; phases of consecutive batches are emitted so the Tile scheduler can
fill PE gaps (projection/o-proj matmuls interleave with the ACT-bound
attention loop, keeping the PE HAM clock-gate at 8/8).

Device layouts (host prepares):
  xT, encT: [B, HIDDEN, S] bf16, wq [HID,256], wkv [HID,128] (k|v cols),
  wo [256, HID] bf16, biases as fp32 columns.
"""

import numpy as np
import ml_dtypes

import concourse.bass as bass
from concourse import bacc
import concourse.mybir as mybir
import concourse.tile as tile
from concourse.bass_utils import run_bass_kernel_spmd
from concourse.masks import make_identity

BF16 = ml_dtypes.bfloat16
F32 = mybir.dt.float32
BF = mybir.dt.bfloat16

B = 2
S = 2048
HID = 2048
D = 64          # head dim
RQ = 4          # query heads per core (per kv group)
CH = RQ * D     # 256 q channels per core
NCORES = 8
NH = HID // 128  # 16 hidden chunks
NST = S // 512   # 4 s-tiles of 512
NKC = S // 128   # 16 key chunks of 128
SCALE = 1.0 / np.sqrt(D)

ID = mybir.ActivationFunctionType.Identity
EXP = mybir.ActivationFunctionType.Exp


def _build_nc() -> bass.Bass:
    nc = bacc.Bacc()

    xT = nc.dram_tensor("xT", [B, HID, S], BF, kind="ExternalInput")
    encT = nc.dram_tensor("encT", [B, HID, S], BF, kind="ExternalInput")
    wq = nc.dram_tensor("wq", [HID, CH], BF, kind="ExternalInput")
    wkv = nc.dram_tensor("wkv", [HID, 128], BF, kind="ExternalInput")
    wo = nc.dram_tensor("wo", [CH, HID], BF, kind="ExternalInput")
    bq = nc.dram_tensor("bq", [CH, 1], F32, kind="ExternalInput")
    bk = nc.dram_tensor("bk", [D, 1], F32, kind="ExternalInput")
    bv = nc.dram_tensor("bv", [D, 1], F32, kind="ExternalInput")
    out = nc.dram_tensor("out", [B, S, HID], BF, kind="ExternalOutput")

    with tile.TileContext(nc) as tc:
        with (
            tc.tile_pool(name="wpool", bufs=1) as wpool,
            tc.tile_pool(name="xs", bufs=8) as xs_pool,
            tc.tile_pool(name="es", bufs=8) as es_pool,
            tc.tile_pool(name="acts", bufs=2) as acts,
            tc.tile_pool(name="vaug", bufs=2) as vaug_pool,
            tc.tile_pool(name="epool", bufs=20) as epool,
            tc.tile_pool(name="small", bufs=2) as small,
            tc.tile_pool(name="osb", bufs=4) as osb_pool,
            tc.tile_pool(name="psum", bufs=2, space="PSUM") as ps,
        ):
            # ---- resident weights ----
            wq_t = []
            wkv_t = []
            for h in range(NH):
                wqh = wpool.tile([128, CH], BF, name=f"wq{h}")
                nc.sync.dma_start(out=wqh[:], in_=wq[h * 128:(h + 1) * 128, :])
                wq_t.append(wqh)
                wkvh = wpool.tile([128, 128], BF, name=f"wkv{h}")
                nc.sync.dma_start(out=wkvh[:], in_=wkv[h * 128:(h + 1) * 128, :])
                wkv_t.append(wkvh)
            wo_t = []
            for cchunk in range(2):
                woc = wpool.tile([128, HID], BF, name=f"wo{cchunk}")
                nc.sync.dma_start(out=woc[:], in_=wo[cchunk * 128:(cchunk + 1) * 128, :])
                wo_t.append(woc)
            bq_t = []
            for cchunk in range(2):
                bqc = wpool.tile([128, 1], F32, name=f"bq{cchunk}")
                nc.sync.dma_start(out=bqc[:], in_=bq[cchunk * 128:(cchunk + 1) * 128, :])
                bq_t.append(bqc)
            bk_t = wpool.tile([D, 1], F32, name="bk_t")
            nc.sync.dma_start(out=bk_t[:], in_=bk[:, :])
            bv_t = wpool.tile([D, 1], F32, name="bv_t")
            nc.sync.dma_start(out=bv_t[:], in_=bv[:, :])

            ident = wpool.tile([128, 128], BF, name="ident")
            make_identity(nc, ident[:])

            # per-batch persistent activation tiles (bufs=2 rotation)
            kdup_b = []
            vt_b = []
            qp_b = []
            otu_b = []
            vaug_b = []
            for b in range(B):
                kdup_b.append(acts.tile([128, S], BF, tag="kdup", name=f"kdup{b}"))
                vt_b.append(acts.tile([D, S], BF, tag="vt", name=f"vt{b}"))
                qp_b.append([
                    acts.tile([128, S], BF, tag=f"qp{p}", name=f"qp{p}_{b}")
                    for p in range(2)
                ])
                otu_b.append([
                    acts.tile([128, S], BF, tag=f"otu{p}", name=f"otu{p}_{b}")
                    for p in range(2)
                ])
                vaug_b.append([
                    vaug_pool.tile([128, D + 1], BF, tag=f"va{kc}", name=f"va{b}{kc}")
                    for kc in range(NKC)
                ])

            def emit_proj(b):
                kdup = kdup_b[b]
                vt = vt_b[b]
                # KV projection (k rows 0:64, v rows 64:128 of psum)
                for st in range(NST):
                    ssl = slice(st * 512, (st + 1) * 512)
                    kvps = ps.tile([128, 512], F32, tag="pproj", name=f"kvps{b}{st}")
                    for h in range(NH):
                        et = es_pool.tile([128, 512], BF, tag="es", name=f"es{b}{st}{h}")
                        nc.gpsimd.dma_start(
                            out=et[:], in_=encT[b, h * 128:(h + 1) * 128, ssl])
                        nc.tensor.matmul(
                            kvps[:], wkv_t[h][:], et[:],
                            start=(h == 0), stop=(h == NH - 1))
                    nc.vector.tensor_scalar_add(
                        kdup[0:D, ssl], kvps[0:D, :], bk_t[:])
                    nc.scalar.activation(
                        kdup[D:128, ssl], kvps[0:D, :], ID, bias=bk_t[:])
                    nc.scalar.activation(
                        vt[0:D, ssl], kvps[D:128, :], ID, bias=bv_t[:])
                # v transposes -> v_aug chunks [128 kpos, 65] with ones col
                for kc in range(NKC):
                    vtp = ps.tile([128, D], BF, tag="pproj", name=f"vtp{b}{kc}")
                    nc.tensor.transpose(
                        vtp[:], vt[:, kc * 128:(kc + 1) * 128], ident[0:D, 0:D])
                    va = vaug_b[b][kc]
                    nc.gpsimd.memset(va[:, D:D + 1], 1.0)
                    nc.vector.tensor_copy(va[:, 0:D], vtp[:])
                # Q projection -> head-pair tiles (heads 2p, 2p+1 stacked)
                for st in range(NST):
                    ssl = slice(st * 512, (st + 1) * 512)
                    qlo = ps.tile([128, 512], F32, tag="pproj", name=f"qlo{b}{st}")
                    qhi = ps.tile([128, 512], F32, tag="pproj", name=f"qhi{b}{st}")
                    for h in range(NH):
                        xt = xs_pool.tile([128, 512], BF, tag="xs", name=f"xs{b}{st}{h}")
                        nc.gpsimd.dma_start(
                            out=xt[:], in_=xT[b, h * 128:(h + 1) * 128, ssl])
                        nc.tensor.matmul(
                            qlo[:], wq_t[h][:, 0:128], xt[:],
                            start=(h == 0), stop=(h == NH - 1))
                        nc.tensor.matmul(
                            qhi[:], wq_t[h][:, 128:256], xt[:],
                            start=(h == 0), stop=(h == NH - 1))
                    nc.vector.tensor_scalar_add(
                        qp_b[b][0][:, ssl], qlo[:], bq_t[0][:])
                    nc.vector.tensor_scalar_add(
                        qp_b[b][1][:, ssl], qhi[:], bq_t[1][:])

            def emit_attn(b):
                kdup = kdup_b[b]
                for pair in range(2):
                    qp = qp_b[b][pair]
                    otu = otu_b[b][pair]
                    for qc in range(NST):
                        qsl = slice(qc * 512, (qc + 1) * 512)
                        avE = ps.tile([D + 1, 512], F32, tag="av",
                                      name=f"avE{b}{pair}{qc}")
                        avO = ps.tile([D + 1, 512], F32, tag="av",
                                      name=f"avO{b}{pair}{qc}")
                        # phase 1: scores + exp for all key chunks (PE stays
                        # in 64x128 row-tiled mode; head 2p on rows 0:64,
                        # head 2p+1 on rows 64:128, concurrent)
                        e_ts = []
                        for kc in range(NKC):
                            ksl = slice(kc * 128, (kc + 1) * 128)
                            sct = ps.tile([128, 1024], F32, tag="sct",
                                          name=f"sct{b}{pair}{qc}{kc}")
                            nc.tensor.matmul(
                                sct[:, 0:512], kdup[0:D, ksl], qp[0:D, qsl],
                                start=True, stop=True)
                            nc.tensor.matmul(
                                sct[:, 512:1024], kdup[D:128, ksl],
                                qp[D:128, qsl], start=True, stop=True)
                            e_t = epool.tile([128, 1024], BF, tag="e",
                                             name=f"e{b}{pair}{qc}{kc}")
                            nc.scalar.activation(
                                e_t[:], sct[:], EXP, scale=float(SCALE))
                            e_ts.append(e_t)
                        # phase 2: AV accumulation (128x128 mode, stationary
                        # v_aug reused for both heads back to back)
                        for kc in range(NKC):
                            va = vaug_b[b][kc]
                            nc.tensor.matmul(
                                avE[:], va[:], e_ts[kc][:, 0:512],
                                start=(kc == 0), stop=(kc == NKC - 1))
                            nc.tensor.matmul(
                                avO[:], va[:], e_ts[kc][:, 512:1024],
                                start=(kc == 0), stop=(kc == NKC - 1))
                        # normalization: row 64 of av holds Z = sum(exp).
                        # Z is moved to partition 0 with an ACT copy first —
                        # recipf/partition_broadcast at partition offsets
                        # mis-read on hardware.
                        for head, av in ((0, avE), (1, avO)):
                            rows = slice(head * D, (head + 1) * D)
                            zr = small.tile([1, 512], F32, tag="zr", bufs=4,
                                            name=f"zr{b}{pair}{qc}{head}")
                            nc.scalar.activation(
                                zr[0:1, :], av[D:D + 1, :], ID)
                            rt = small.tile([1, 512], F32, tag="rt", bufs=4,
                                            name=f"rt{b}{pair}{qc}{head}")
                            nc.vector.reciprocal_approx_fast(
                                rt[0:1, :], zr[0:1, :])
                            rb = small.tile([128, 512], F32, tag="rb", bufs=4,
                                            name=f"rb{b}{pair}{qc}{head}")
                            nc.gpsimd.partition_broadcast(rb[:], rt[0:1, :])
                            if head == 0:
                                nc.vector.tensor_mul(
                                    otu[rows, qsl], av[0:D, :], rb[0:D, :])
                            else:
                                avs = small.tile([128, 512], F32, tag="avsb",
                                                 name=f"avs{b}{pair}{qc}")
                                nc.scalar.activation(
                                    avs[D:128, :], av[0:D, :], ID)
                                nc.vector.tensor_mul(
                                    otu[rows, qsl], avs[D:128, :],
                                    rb[D:128, :])

            def emit_oproj(b):
                for sc16 in range(S // 128):
                    s128 = slice(sc16 * 128, (sc16 + 1) * 128)
                    for hcp in range(2):
                        opa = ps.tile([128, 512], F32, tag="pproj",
                                      name=f"opa{b}{sc16}{hcp}")
                        opb = ps.tile([128, 512], F32, tag="pproj",
                                      name=f"opb{b}{sc16}{hcp}")
                        hsl_a = slice((2 * hcp) * 512, (2 * hcp + 1) * 512)
                        hsl_b = slice((2 * hcp + 1) * 512, (2 * hcp + 2) * 512)
                        # group by stationary (otu chunk) to amortize LDW
                        nc.tensor.matmul(
                            opa[:], otu_b[b][0][:, s128], wo_t[0][:, hsl_a],
                            start=True, stop=False)
                        nc.tensor.matmul(
                            opb[:], otu_b[b][0][:, s128], wo_t[0][:, hsl_b],
                            start=True, stop=False)
                        nc.tensor.matmul(
                            opa[:], otu_b[b][1][:, s128], wo_t[1][:, hsl_a],
                            start=False, stop=True)
                        nc.tensor.matmul(
                            opb[:], otu_b[b][1][:, s128], wo_t[1][:, hsl_b],
                            start=False, stop=True)
                        for op, hsl in ((opa, hsl_a), (opb, hsl_b)):
                            ob = osb_pool.tile([128, 512], BF, tag="osb",
                                               name=f"ob{b}{sc16}{hsl.start}")
                            nc.vector.tensor_copy(ob[:], op[:])
                            nc.sync.dma_start(out=out[b, s128, hsl], in_=ob[:])

            # emission order: P0 A0 P1 O0 A1 O1 — the tile scheduler fills
            # PE gaps of the ACT-bound attention with proj/o-proj matmuls.
            emit_proj(0)
            emit_attn(0)
            emit_proj(1)
            emit_oproj(0)
            emit_attn(1)
            emit_oproj(1)

    if not nc.is_finalized():
        nc.finalize()
    return nc


_NC = None
_RUN_KWARGS = {}
_LAST_RESULT = None


def _get_nc():
    global _NC
    if _NC is None:
        _NC = _build_nc()
    return _NC


def kernel(x, encoder_output, Wq, bq, Wk, bk, Wv, bv, Wo, bo):
    nc = _get_nc()
    xT = np.ascontiguousarray(np.asarray(x, np.float32).transpose(0, 2, 1)).astype(BF16)
    encT = np.ascontiguousarray(
        np.asarray(encoder_output, np.float32).transpose(0, 2, 1)).astype(BF16)
    Wq = np.asarray(Wq, np.float32)
    Wk = np.asarray(Wk, np.float32)
    Wv = np.asarray(Wv, np.float32)
    Wo = np.asarray(Wo, np.float32)
    in_maps = []
    for c in range(NCORES):
        csl = slice(c * CH, (c + 1) * CH)
        gsl = slice(c * D, (c + 1) * D)
        wkv_c = np.concatenate([Wk[:, gsl], Wv[:, gsl]], axis=1)
        in_maps.append({
            "xT": xT,
            "encT": encT,
            "wq": np.ascontiguousarray(Wq[:, csl]).astype(BF16),
            "wkv": np.ascontiguousarray(wkv_c).astype(BF16),
            "wo": np.ascontiguousarray(Wo[csl, :]).astype(BF16),
            "bq": np.ascontiguousarray(
                np.asarray(bq, np.float32)[csl].reshape(CH, 1)),
            "bk": np.ascontiguousarray(
                np.asarray(bk, np.float32)[gsl].reshape(D, 1)),
            "bv": np.ascontiguousarray(
                np.asarray(bv, np.float32)[gsl].reshape(D, 1)),
        })
    res = run_bass_kernel_spmd(nc, in_maps, list(range(NCORES)), **_RUN_KWARGS)
    global _LAST_RESULT
    _LAST_RESULT = res
    total = np.zeros((B, S, HID), np.float32)
    for c in range(NCORES):
        total += res.results[c]["out"].astype(np.float32)
    return total + np.asarray(bo, np.float32)


# revision 14
# speedup vs baseline: 2.1209x; 1.0718x over previous
"""GQA cross-attention block on 8 trn2 NeuronCores (v2).

Sharding: tensor-parallel over heads. Core c owns KV group g=c (64 dims of
K/V) and its 4 query heads (256 q channels). Each core computes its heads'
attention plus its slice of the o-projection (rows c*256:(c+1)*256 of Wo),
producing a full-shape partial output; the host sums the 8 partials and
adds bo. No device collectives needed.

v2 schedule (vs v1): K|V projection fused into one M=128 matmul chain;
scores row-tiled via tile_position so two heads' K=64 matmuls share the PE
array; exp runs on [128,1024] two-bank PSUM spans (half the ACT instruction
overhead); softmax normalization uses reciprocal_approx_fast + gpsimd
partition_broadcast instead of the 1-partition DVE reciprocal + ones-matmul
chain; phases of consecutive batches are emitted so the Tile scheduler can
fill PE gaps (projection/o-proj matmuls interleave with the ACT-bound
attention loop, keeping the PE HAM clock-gate at 8/8).

Device layouts (host prepares):
  xT, encT: [B, HIDDEN, S] bf16, wq [HID,256], wkv [HID,128] (k|v cols),
  wo [256, HID] bf16, biases as fp32 columns.
"""

import numpy as np
import ml_dtypes

import concourse.bass as bass
from concourse import bacc
import concourse.mybir as mybir
import concourse.tile as tile
from concourse.bass_utils import run_bass_kernel_spmd
from concourse.masks import make_identity

BF16 = ml_dtypes.bfloat16
F32 = mybir.dt.float32
BF = mybir.dt.bfloat16

B = 2
S = 2048
HID = 2048
D = 64          # head dim
RQ = 4          # query heads per core (per kv group)
CH = RQ * D     # 256 q channels per core
NCORES = 8
NH = HID // 128  # 16 hidden chunks
NST = S // 512   # 4 s-tiles of 512
NKC = S // 128   # 16 key chunks of 128
SCALE = 1.0 / np.sqrt(D)

ID = mybir.ActivationFunctionType.Identity
EXP = mybir.ActivationFunctionType.Exp


def _build_nc() -> bass.Bass:
    nc = bacc.Bacc()

    xT = nc.dram_tensor("xT", [B, HID, S], BF, kind="ExternalInput")
    encT = nc.dram_tensor("encT", [B, HID, S], BF, kind="ExternalInput")
    wq = nc.dram_tensor("wq", [HID, CH], BF, kind="ExternalInput")
    wkv = nc.dram_tensor("wkv", [HID, 128], BF, kind="ExternalInput")
    wo = nc.dram_tensor("wo", [CH, HID], BF, kind="ExternalInput")
    bq = nc.dram_tensor("bq", [CH, 1], F32, kind="ExternalInput")
    bk = nc.dram_tensor("bk", [D, 1], F32, kind="ExternalInput")
    bv = nc.dram_tensor("bv", [D, 1], F32, kind="ExternalInput")
    out = nc.dram_tensor("out", [B, S, HID], BF, kind="ExternalOutput")

    with tile.TileContext(nc) as tc:
        with (
            tc.tile_pool(name="wpool", bufs=1) as wpool,
            tc.tile_pool(name="xs", bufs=20) as xs_pool,
            tc.tile_pool(name="es", bufs=20) as es_pool,
            tc.tile_pool(name="acts", bufs=2) as acts,
            tc.tile_pool(name="vaug", bufs=2) as vaug_pool,
            tc.tile_pool(name="epool", bufs=4) as epool,
            tc.tile_pool(name="small", bufs=2) as small,
            tc.tile_pool(name="osb", bufs=4) as osb_pool,
            tc.tile_pool(name="psum", bufs=2, space="PSUM") as ps,
        ):
            # ---- resident weights ----
            wq_t = []
            wkv_t = []
            for h in range(NH):
                wqh = wpool.tile([128, CH], BF, name=f"wq{h}")
                nc.sync.dma_start(out=wqh[:], in_=wq[h * 128:(h + 1) * 128, :])
                wq_t.append(wqh)
                wkvh = wpool.tile([128, 128], BF, name=f"wkv{h}")
                nc.sync.dma_start(out=wkvh[:], in_=wkv[h * 128:(h + 1) * 128, :])
                wkv_t.append(wkvh)
            wo_t = []
            for cchunk in range(2):
                woc = wpool.tile([128, HID], BF, name=f"wo{cchunk}")
                nc.sync.dma_start(out=woc[:], in_=wo[cchunk * 128:(cchunk + 1) * 128, :])
                wo_t.append(woc)
            bq_t = []
            for cchunk in range(2):
                bqc = wpool.tile([128, 1], F32, name=f"bq{cchunk}")
                nc.sync.dma_start(out=bqc[:], in_=bq[cchunk * 128:(cchunk + 1) * 128, :])
                bq_t.append(bqc)
            bk_t = wpool.tile([D, 1], F32, name="bk_t")
            nc.sync.dma_start(out=bk_t[:], in_=bk[:, :])
            bv_t = wpool.tile([D, 1], F32, name="bv_t")
            nc.sync.dma_start(out=bv_t[:], in_=bv[:, :])

            ident = wpool.tile([128, 128], BF, name="ident")
            make_identity(nc, ident[:])

            # per-batch persistent activation tiles (bufs=2 rotation)
            kdup_b = []
            vt_b = []
            qp_b = []
            otu_b = []
            vaug_b = []
            for b in range(B):
                kdup_b.append(acts.tile([128, S], BF, tag="kdup", name=f"kdup{b}"))
                vt_b.append(acts.tile([D, S], BF, tag="vt", name=f"vt{b}"))
                qp_b.append([
                    acts.tile([128, S], BF, tag=f"qp{p}", name=f"qp{p}_{b}")
                    for p in range(2)
                ])
                otu_b.append([
                    acts.tile([128, S], BF, tag=f"otu{p}", name=f"otu{p}_{b}")
                    for p in range(2)
                ])
                vaug_b.append([
                    vaug_pool.tile([128, 96], BF, tag=f"va{kc}", name=f"va{b}{kc}")
                    for kc in range(NKC)
                ])

            def emit_proj(b):
                kdup = kdup_b[b]
                vt = vt_b[b]
                # KV projection (k rows 0:64, v rows 64:128 of psum); st in
                # pairs with the stationary wkv chunk reused for both s-tiles
                for stp in range(NST // 2):
                    st0, st1 = 2 * stp, 2 * stp + 1
                    kv0 = ps.tile([128, 512], F32, tag="pproj", name=f"kvps{b}{st0}")
                    kv1 = ps.tile([128, 512], F32, tag="pproj", name=f"kvps{b}{st1}")
                    psl = slice(st0 * 512, (st1 + 1) * 512)
                    for h in range(NH):
                        et = es_pool.tile([128, 1024], BF, tag="es",
                                          name=f"es{b}{stp}{h}")
                        nc.gpsimd.dma_start(
                            out=et[:], in_=encT[b, h * 128:(h + 1) * 128, psl])
                        nc.tensor.matmul(
                            kv0[:], wkv_t[h][:], et[:, 0:512],
                            start=(h == 0), stop=(h == NH - 1))
                        nc.tensor.matmul(
                            kv1[:], wkv_t[h][:], et[:, 512:1024],
                            start=(h == 0), stop=(h == NH - 1))
                    for st, kvps in ((st0, kv0), (st1, kv1)):
                        ssl = slice(st * 512, (st + 1) * 512)
                        nc.vector.tensor_scalar_add(
                            kdup[0:D, ssl], kvps[0:D, :], bk_t[:])
                        nc.scalar.activation(
                            kdup[D:128, ssl], kvps[0:D, :], ID, bias=bk_t[:])
                        nc.scalar.activation(
                            vt[0:D, ssl], kvps[D:128, :], ID, bias=bv_t[:])
                # v transposes -> v_aug chunks [128 kpos, 65] with ones col
                for kc in range(NKC):
                    vtp = ps.tile([128, D], BF, tag="pproj", name=f"vtp{b}{kc}")
                    nc.tensor.transpose(
                        vtp[:], vt[:, kc * 128:(kc + 1) * 128], ident[0:D, 0:D])
                    va = vaug_b[b][kc]
                    nc.gpsimd.memset(va[:, D:D + 1], 1.0)
                    nc.gpsimd.memset(va[:, D + 1:96], 0.0)
                    nc.vector.tensor_copy(va[:, 0:D], vtp[:])
                # Q projection -> head-pair tiles (heads 2p, 2p+1 stacked).
                # st processed in pairs with the stationary weight reused for
                # both s-tiles (halves LDWEIGHTS pressure); lo/hi in separate
                # sweeps so only 2 PSUM banks are live.
                for stp in range(NST // 2):
                    st0, st1 = 2 * stp, 2 * stp + 1
                    ssl0 = slice(st0 * 512, (st0 + 1) * 512)
                    ssl1 = slice(st1 * 512, (st1 + 1) * 512)
                    psl = slice(st0 * 512, (st1 + 1) * 512)
                    xts = []
                    for h in range(NH):
                        xt = xs_pool.tile([128, 1024], BF, tag="xs",
                                          name=f"xs{b}{stp}{h}")
                        nc.gpsimd.dma_start(
                            out=xt[:], in_=xT[b, h * 128:(h + 1) * 128, psl])
                        xts.append(xt)
                    for half, bias_t, qdst in (
                        (slice(0, 128), bq_t[0], qp_b[b][0]),
                        (slice(128, 256), bq_t[1], qp_b[b][1]),
                    ):
                        q0 = ps.tile([128, 512], F32, tag="pproj",
                                     name=f"q{half.start}{b}{st0}")
                        q1 = ps.tile([128, 512], F32, tag="pproj",
                                     name=f"q{half.start}{b}{st1}")
                        for h in range(NH):
                            nc.tensor.matmul(
                                q0[:], wq_t[h][:, half], xts[h][:, 0:512],
                                start=(h == 0), stop=(h == NH - 1))
                            nc.tensor.matmul(
                                q1[:], wq_t[h][:, half], xts[h][:, 512:1024],
                                start=(h == 0), stop=(h == NH - 1))
                        nc.vector.tensor_scalar_add(qdst[:, ssl0], q0[:], bias_t[:])
                        nc.vector.tensor_scalar_add(qdst[:, ssl1], q1[:], bias_t[:])

            def emit_attn(b):
                kdup = kdup_b[b]
                for pair in range(2):
                    qp = qp_b[b][pair]
                    otu = otu_b[b][pair]
                    for qc in range(NST):
                        qsl = slice(qc * 512, (qc + 1) * 512)
                        avE = ps.tile([128, 512], F32, tag="av",
                                      name=f"avE{b}{pair}{qc}")
                        avO = ps.tile([128, 512], F32, tag="av",
                                      name=f"avO{b}{pair}{qc}")
                        for kc in range(NKC):
                            ksl = slice(kc * 128, (kc + 1) * 128)
                            sct = ps.tile([128, 1024], F32, tag="sct",
                                          name=f"sct{b}{pair}{qc}{kc}")
                            # row-tiled pair: head 2p on PE rows 0:64,
                            # head 2p+1 on rows 64:128 (concurrent)
                            nc.tensor.matmul(
                                sct[:, 0:512], kdup[0:D, ksl], qp[0:D, qsl],
                                start=True, stop=True)
                            nc.tensor.matmul(
                                sct[:, 512:1024], kdup[D:128, ksl],
                                qp[D:128, qsl], start=True, stop=True)
                            e_t = epool.tile([128, 1024], BF, tag="e",
                                             name=f"e{b}{pair}{qc}{kc}")
                            nc.scalar.activation(
                                e_t[:], sct[:], EXP, scale=float(SCALE))
                            va = vaug_b[b][kc]
                            nc.tensor.matmul(
                                avE[0:96, :], va[:], e_t[:, 0:512],
                                start=(kc == 0), stop=(kc == NKC - 1))
                            nc.tensor.matmul(
                                avO[0:96, :], va[:], e_t[:, 512:1024],
                                start=(kc == 0), stop=(kc == NKC - 1))
                        # normalization: row 64 of av holds Z = sum(exp).
                        # Z moves to partition 0 via stream_shuffle (the only
                        # legal cross-quadrant DVE move; recip/broadcast at
                        # partition offsets mis-read on hardware).
                        IDMASK = list(range(32))
                        for head, av in ((0, avE), (1, avO)):
                            rows = slice(head * D, (head + 1) * D)
                            zs = small.tile([32, 512], F32, tag="zr", bufs=4,
                                            name=f"zr{b}{pair}{qc}{head}")
                            nc.vector.stream_shuffle(
                                zs[0:32, :], av[D:D + 32, :], IDMASK)
                            rt = small.tile([1, 512], F32, tag="rt", bufs=4,
                                            name=f"rt{b}{pair}{qc}{head}")
                            nc.vector.reciprocal_approx_fast(
                                rt[0:1, :], zs[0:1, :])
                            rb = small.tile([128, 512], F32, tag="rb", bufs=4,
                                            name=f"rb{b}{pair}{qc}{head}")
                            nc.gpsimd.partition_broadcast(rb[:], rt[0:1, :])
                            if head == 0:
                                avc = small.tile([D, 512], F32, tag="avc",
                                                 bufs=2, name=f"avc{b}{pair}{qc}")
                                nc.vector.tensor_copy(avc[:], av[0:D, :])
                                nc.vector.tensor_mul(
                                    otu[rows, qsl], avc[:], rb[0:D, :])
                            else:
                                avs = small.tile([128, 512], F32, tag="avsb",
                                                 name=f"avs{b}{pair}{qc}")
                                nc.vector.stream_shuffle(
                                    avs[D:D + 32, :], av[0:32, :], IDMASK)
                                nc.vector.stream_shuffle(
                                    avs[D + 32:128, :], av[32:D, :], IDMASK)
                                nc.vector.tensor_mul(
                                    otu[rows, qsl], avs[D:128, :],
                                    rb[D:128, :])

            def emit_oproj(b):
                for sc16 in range(S // 128):
                    s128 = slice(sc16 * 128, (sc16 + 1) * 128)
                    for hcp in range(2):
                        opa = ps.tile([128, 512], F32, tag="pproj",
                                      name=f"opa{b}{sc16}{hcp}")
                        opb = ps.tile([128, 512], F32, tag="pproj",
                                      name=f"opb{b}{sc16}{hcp}")
                        hsl_a = slice((2 * hcp) * 512, (2 * hcp + 1) * 512)
                        hsl_b = slice((2 * hcp + 1) * 512, (2 * hcp + 2) * 512)
                        # group by stationary (otu chunk) to amortize LDW
                        nc.tensor.matmul(
                            opa[:], otu_b[b][0][:, s128], wo_t[0][:, hsl_a],
                            start=True, stop=False)
                        nc.tensor.matmul(
                            opb[:], otu_b[b][0][:, s128], wo_t[0][:, hsl_b],
                            start=True, stop=False)
                        nc.tensor.matmul(
                            opa[:], otu_b[b][1][:, s128], wo_t[1][:, hsl_a],
                            start=False, stop=True)
                        nc.tensor.matmul(
                            opb[:], otu_b[b][1][:, s128], wo_t[1][:, hsl_b],
                            start=False, stop=True)
                        for op, hsl in ((opa, hsl_a), (opb, hsl_b)):
                            ob = osb_pool.tile([128, 512], BF, tag="osb",
                                               name=f"ob{b}{sc16}{hsl.start}")
                            nc.vector.tensor_copy(ob[:], op[:])
                            nc.sync.dma_start(out=out[b, s128, hsl], in_=ob[:])

            # emission order: P0 A0 P1 O0 A1 O1 — the tile scheduler fills
            # PE gaps of the ACT-bound attention with proj/o-proj matmuls.
            emit_proj(0)
            emit_attn(0)
            emit_proj(1)
            emit_oproj(0)
            emit_attn(1)
            emit_oproj(1)

    if not nc.is_finalized():
        nc.finalize()
    return nc


_NC = None
_RUN_KWARGS = {}
_LAST_RESULT = None


def _get_nc():
    global _NC
    if _NC is None:
        _NC = _build_nc()
    return _NC


def kernel(x, encoder_output, Wq, bq, Wk, bk, Wv, bv, Wo, bo):
    nc = _get_nc()
    xT = np.ascontiguousarray(np.asarray(x, np.float32).transpose(0, 2, 1)).astype(BF16)
    encT = np.ascontiguousarray(
        np.asarray(encoder_output, np.float32).transpose(0, 2, 1)).astype(BF16)
    Wq = np.asarray(Wq, np.float32)
    Wk = np.asarray(Wk, np.float32)
    Wv = np.asarray(Wv, np.float32)
    Wo = np.asarray(Wo, np.float32)
    in_maps = []
    for c in range(NCORES):
        csl = slice(c * CH, (c + 1) * CH)
        gsl = slice(c * D, (c + 1) * D)
        wkv_c = np.concatenate([Wk[:, gsl], Wv[:, gsl]], axis=1)
        in_maps.append({
            "xT": xT,
            "encT": encT,
            "wq": np.ascontiguousarray(Wq[:, csl]).astype(BF16),
            "wkv": np.ascontiguousarray(wkv_c).astype(BF16),
            "wo": np.ascontiguousarray(Wo[csl, :]).astype(BF16),
            "bq": np.ascontiguousarray(
                np.asarray(bq, np.float32)[csl].reshape(CH, 1)),
            "bk": np.ascontiguousarray(
                np.asarray(bk, np.float32)[gsl].reshape(D, 1)),
            "bv": np.ascontiguousarray(
                np.asarray(bv, np.float32)[gsl].reshape(D, 1)),
        })
    res = run_bass_kernel_spmd(nc, in_maps, list(range(NCORES)), **_RUN_KWARGS)
    global _LAST_RESULT
    _LAST_RESULT = res
    total = np.zeros((B, S, HID), np.float32)
    for c in range(NCORES):
        total += res.results[c]["out"].astype(np.float32)
    return total + np.asarray(bo, np.float32)


# revision 15
# speedup vs baseline: 2.1675x; 1.0220x over previous
"""GQA cross-attention block on 8 trn2 NeuronCores (v2).

Sharding: tensor-parallel over heads. Core c owns KV group g=c (64 dims of
K/V) and its 4 query heads (256 q channels). Each core computes its heads'
attention plus its slice of the o-projection (rows c*256:(c+1)*256 of Wo),
producing a full-shape partial output; the host sums the 8 partials and
adds bo. No device collectives needed.

v2 schedule (vs v1): K|V projection fused into one M=128 matmul chain;
scores row-tiled via tile_position so two heads' K=64 matmuls share the PE
array; exp runs on [128,1024] two-bank PSUM spans (half the ACT instruction
overhead); softmax normalization uses reciprocal_approx_fast + gpsimd
partition_broadcast instead of the 1-partition DVE reciprocal + ones-matmul
chain; phases of consecutive batches are emitted so the Tile scheduler can
fill PE gaps (projection/o-proj matmuls interleave with the ACT-bound
attention loop, keeping the PE HAM clock-gate at 8/8).

Device layouts (host prepares):
  xT, encT: [B, HIDDEN, S] bf16, wq [HID,256], wkv [HID,128] (k|v cols),
  wo [256, HID] bf16, biases as fp32 columns.
"""

import numpy as np
import ml_dtypes

import concourse.bass as bass
from concourse import bacc
import concourse.mybir as mybir
import concourse.tile as tile
from concourse.bass_utils import run_bass_kernel_spmd
from concourse.masks import make_identity

BF16 = ml_dtypes.bfloat16
F32 = mybir.dt.float32
BF = mybir.dt.bfloat16

B = 2
S = 2048
HID = 2048
D = 64          # head dim
RQ = 4          # query heads per core (per kv group)
CH = RQ * D     # 256 q channels per core
NCORES = 8
NH = HID // 128  # 16 hidden chunks
NST = S // 512   # 4 s-tiles of 512
NKC = S // 128   # 16 key chunks of 128
SCALE = 1.0 / np.sqrt(D)

ID = mybir.ActivationFunctionType.Identity
EXP = mybir.ActivationFunctionType.Exp


def _build_nc() -> bass.Bass:
    nc = bacc.Bacc()

    xT = nc.dram_tensor("xT", [B, HID, S], BF, kind="ExternalInput")
    encT = nc.dram_tensor("encT", [B, HID, S], BF, kind="ExternalInput")
    wq = nc.dram_tensor("wq", [HID, CH], BF, kind="ExternalInput")
    wkv = nc.dram_tensor("wkv", [HID, 128], BF, kind="ExternalInput")
    wo = nc.dram_tensor("wo", [CH, HID], BF, kind="ExternalInput")
    bq = nc.dram_tensor("bq", [CH, 1], F32, kind="ExternalInput")
    bk = nc.dram_tensor("bk", [D, 1], F32, kind="ExternalInput")
    bv = nc.dram_tensor("bv", [D, 1], F32, kind="ExternalInput")
    out = nc.dram_tensor("out", [B, S, HID], BF, kind="ExternalOutput")

    with tile.TileContext(nc) as tc:
        with (
            tc.tile_pool(name="wpool", bufs=1) as wpool,
            tc.tile_pool(name="xs", bufs=20) as xs_pool,
            tc.tile_pool(name="es", bufs=20) as es_pool,
            tc.tile_pool(name="acts", bufs=2) as acts,
            tc.tile_pool(name="vaug", bufs=2) as vaug_pool,
            tc.tile_pool(name="epool", bufs=4) as epool,
            tc.tile_pool(name="small", bufs=2) as small,
            tc.tile_pool(name="osb", bufs=4) as osb_pool,
            tc.tile_pool(name="psum", bufs=2, space="PSUM") as ps,
        ):
            # ---- resident weights ----
            wq_t = []
            wkv_t = []
            for h in range(NH):
                wqh = wpool.tile([128, CH], BF, name=f"wq{h}")
                nc.sync.dma_start(out=wqh[:], in_=wq[h * 128:(h + 1) * 128, :])
                wq_t.append(wqh)
                wkvh = wpool.tile([128, 128], BF, name=f"wkv{h}")
                nc.sync.dma_start(out=wkvh[:], in_=wkv[h * 128:(h + 1) * 128, :])
                wkv_t.append(wkvh)
            wo_t = []
            for cchunk in range(2):
                woc = wpool.tile([128, HID], BF, name=f"wo{cchunk}")
                nc.sync.dma_start(out=woc[:], in_=wo[cchunk * 128:(cchunk + 1) * 128, :])
                wo_t.append(woc)
            bq_t = []
            for cchunk in range(2):
                bqc = wpool.tile([128, 1], F32, name=f"bq{cchunk}")
                nc.sync.dma_start(out=bqc[:], in_=bq[cchunk * 128:(cchunk + 1) * 128, :])
                bq_t.append(bqc)
            bk_t = wpool.tile([D, 1], F32, name="bk_t")
            nc.sync.dma_start(out=bk_t[:], in_=bk[:, :])
            bv_t = wpool.tile([D, 1], F32, name="bv_t")
            nc.sync.dma_start(out=bv_t[:], in_=bv[:, :])

            ident = wpool.tile([128, 128], BF, name="ident")
            make_identity(nc, ident[:])

            # per-batch persistent activation tiles (bufs=2 rotation)
            kdup_b = []
            vt_b = []
            qp_b = []
            otu_b = []
            vaug_b = []
            for b in range(B):
                kdup_b.append(acts.tile([128, S], BF, tag="kdup", name=f"kdup{b}"))
                vt_b.append(acts.tile([D, S], BF, tag="vt", name=f"vt{b}"))
                qp_b.append([
                    acts.tile([128, S], BF, tag=f"qp{p}", name=f"qp{p}_{b}")
                    for p in range(2)
                ])
                otu_b.append([
                    acts.tile([128, S], BF, tag=f"otu{p}", name=f"otu{p}_{b}")
                    for p in range(2)
                ])
                vaug_b.append([
                    vaug_pool.tile([128, 96], BF, tag=f"va{kc}", name=f"va{b}{kc}")
                    for kc in range(NKC)
                ])

            def emit_proj(b):
                kdup = kdup_b[b]
                vt = vt_b[b]
                # KV projection (k rows 0:64, v rows 64:128 of psum); st in
                # pairs with the stationary wkv chunk reused for both s-tiles
                for stp in range(NST // 2):
                    st0, st1 = 2 * stp, 2 * stp + 1
                    kv0 = ps.tile([128, 512], F32, tag="pproj", name=f"kvps{b}{st0}")
                    kv1 = ps.tile([128, 512], F32, tag="pproj", name=f"kvps{b}{st1}")
                    psl = slice(st0 * 512, (st1 + 1) * 512)
                    for h in range(NH):
                        et = es_pool.tile([128, 1024], BF, tag="es",
                                          name=f"es{b}{stp}{h}")
                        nc.sync.dma_start(
                            out=et[:], in_=encT[b, h * 128:(h + 1) * 128, psl])
                        nc.tensor.matmul(
                            kv0[:], wkv_t[h][:], et[:, 0:512],
                            start=(h == 0), stop=(h == NH - 1))
                        nc.tensor.matmul(
                            kv1[:], wkv_t[h][:], et[:, 512:1024],
                            start=(h == 0), stop=(h == NH - 1))
                    for st, kvps in ((st0, kv0), (st1, kv1)):
                        ssl = slice(st * 512, (st + 1) * 512)
                        nc.vector.tensor_scalar_add(
                            kdup[0:D, ssl], kvps[0:D, :], bk_t[:])
                        nc.scalar.activation(
                            kdup[D:128, ssl], kvps[0:D, :], ID, bias=bk_t[:])
                        nc.scalar.activation(
                            vt[0:D, ssl], kvps[D:128, :], ID, bias=bv_t[:])
                # v transposes -> v_aug chunks [128 kpos, 65] with ones col
                for kc in range(NKC):
                    vtp = ps.tile([128, D], BF, tag="pproj", name=f"vtp{b}{kc}")
                    nc.tensor.transpose(
                        vtp[:], vt[:, kc * 128:(kc + 1) * 128], ident[0:D, 0:D])
                    va = vaug_b[b][kc]
                    nc.gpsimd.memset(va[:, D:D + 1], 1.0)
                    nc.gpsimd.memset(va[:, D + 1:96], 0.0)
                    nc.vector.tensor_copy(va[:, 0:D], vtp[:])
                # Q projection -> head-pair tiles (heads 2p, 2p+1 stacked).
                # st processed in pairs with the stationary weight reused for
                # both s-tiles (halves LDWEIGHTS pressure); lo/hi in separate
                # sweeps so only 2 PSUM banks are live.
                for stp in range(NST // 2):
                    st0, st1 = 2 * stp, 2 * stp + 1
                    ssl0 = slice(st0 * 512, (st0 + 1) * 512)
                    ssl1 = slice(st1 * 512, (st1 + 1) * 512)
                    psl = slice(st0 * 512, (st1 + 1) * 512)
                    xts = []
                    for h in range(NH):
                        xt = xs_pool.tile([128, 1024], BF, tag="xs",
                                          name=f"xs{b}{stp}{h}")
                        nc.gpsimd.dma_start(
                            out=xt[:], in_=xT[b, h * 128:(h + 1) * 128, psl])
                        xts.append(xt)
                    for half, bias_t, qdst in (
                        (slice(0, 128), bq_t[0], qp_b[b][0]),
                        (slice(128, 256), bq_t[1], qp_b[b][1]),
                    ):
                        q0 = ps.tile([128, 512], F32, tag="pproj",
                                     name=f"q{half.start}{b}{st0}")
                        q1 = ps.tile([128, 512], F32, tag="pproj",
                                     name=f"q{half.start}{b}{st1}")
                        for h in range(NH):
                            nc.tensor.matmul(
                                q0[:], wq_t[h][:, half], xts[h][:, 0:512],
                                start=(h == 0), stop=(h == NH - 1))
                            nc.tensor.matmul(
                                q1[:], wq_t[h][:, half], xts[h][:, 512:1024],
                                start=(h == 0), stop=(h == NH - 1))
                        nc.vector.tensor_scalar_add(qdst[:, ssl0], q0[:], bias_t[:])
                        nc.vector.tensor_scalar_add(qdst[:, ssl1], q1[:], bias_t[:])

            def emit_attn(b, inline_oproj=False):
                kdup = kdup_b[b]
                for qc in range(NST):
                    qsl = slice(qc * 512, (qc + 1) * 512)
                    for pair in range(2):
                        qp = qp_b[b][pair]
                        otu = otu_b[b][pair]
                        avE = ps.tile([128, 512], F32, tag="av",
                                      name=f"avE{b}{pair}{qc}")
                        avO = ps.tile([128, 512], F32, tag="av",
                                      name=f"avO{b}{pair}{qc}")
                        for kc in range(NKC):
                            ksl = slice(kc * 128, (kc + 1) * 128)
                            sct = ps.tile([128, 1024], F32, tag="sct",
                                          name=f"sct{b}{pair}{qc}{kc}")
                            # row-tiled pair: head 2p on PE rows 0:64,
                            # head 2p+1 on rows 64:128 (concurrent)
                            nc.tensor.matmul(
                                sct[:, 0:512], kdup[0:D, ksl], qp[0:D, qsl],
                                start=True, stop=True)
                            nc.tensor.matmul(
                                sct[:, 512:1024], kdup[D:128, ksl],
                                qp[D:128, qsl], start=True, stop=True)
                            e_t = epool.tile([128, 1024], BF, tag="e",
                                             name=f"e{b}{pair}{qc}{kc}")
                            nc.scalar.activation(
                                e_t[:], sct[:], EXP, scale=float(SCALE))
                            va = vaug_b[b][kc]
                            nc.tensor.matmul(
                                avE[0:96, :], va[:], e_t[:, 0:512],
                                start=(kc == 0), stop=(kc == NKC - 1))
                            nc.tensor.matmul(
                                avO[0:96, :], va[:], e_t[:, 512:1024],
                                start=(kc == 0), stop=(kc == NKC - 1))
                        # --- evacuate PSUM first (frees av banks for the
                        # next pass), then the normalization chains ---
                        IDMASK = list(range(32))
                        zsE = small.tile([32, 512], F32, tag="zr", bufs=4,
                                         name=f"zrE{b}{pair}{qc}")
                        nc.vector.stream_shuffle(
                            zsE[0:32, :], avE[D:D + 32, :], IDMASK)
                        avc = small.tile([D, 512], F32, tag="avc",
                                         bufs=2, name=f"avc{b}{pair}{qc}")
                        nc.vector.tensor_copy(avc[:], avE[0:D, :])
                        zsO = small.tile([32, 512], F32, tag="zr", bufs=4,
                                         name=f"zrO{b}{pair}{qc}")
                        nc.vector.stream_shuffle(
                            zsO[0:32, :], avO[D:D + 32, :], IDMASK)
                        avs = small.tile([128, 512], F32, tag="avsb",
                                         name=f"avs{b}{pair}{qc}")
                        nc.vector.stream_shuffle(
                            avs[D:D + 32, :], avO[0:32, :], IDMASK)
                        nc.vector.stream_shuffle(
                            avs[D + 32:128, :], avO[32:D, :], IDMASK)
                        # normalization (off the PSUM critical path)
                        for head, zs, src_ap, rows in (
                            (0, zsE, avc[:], slice(0, D)),
                            (1, zsO, avs[D:128, :], slice(D, 128)),
                        ):
                            rt = small.tile([1, 512], F32, tag="rt", bufs=4,
                                            name=f"rt{b}{pair}{qc}{head}")
                            nc.vector.reciprocal_approx_fast(
                                rt[0:1, :], zs[0:1, :])
                            rb = small.tile([128, 512], F32, tag="rb", bufs=4,
                                            name=f"rb{b}{pair}{qc}{head}")
                            nc.gpsimd.partition_broadcast(rb[:], rt[0:1, :])
                            nc.vector.tensor_mul(
                                otu[rows, qsl],
                                src_ap, rb[rows, :])
                    if inline_oproj:
                        emit_oproj_qc(b, qc)

            def emit_oproj_qc(b, qc):
                for sc16 in range(4 * qc, 4 * qc + 4):
                    s128 = slice(sc16 * 128, (sc16 + 1) * 128)
                    for hcp in range(2):
                        opa = ps.tile([128, 512], F32, tag="pproj",
                                      name=f"opa{b}{sc16}{hcp}")
                        opb = ps.tile([128, 512], F32, tag="pproj",
                                      name=f"opb{b}{sc16}{hcp}")
                        hsl_a = slice((2 * hcp) * 512, (2 * hcp + 1) * 512)
                        hsl_b = slice((2 * hcp + 1) * 512, (2 * hcp + 2) * 512)
                        # group by stationary (otu chunk) to amortize LDW
                        nc.tensor.matmul(
                            opa[:], otu_b[b][0][:, s128], wo_t[0][:, hsl_a],
                            start=True, stop=False)
                        nc.tensor.matmul(
                            opb[:], otu_b[b][0][:, s128], wo_t[0][:, hsl_b],
                            start=True, stop=False)
                        nc.tensor.matmul(
                            opa[:], otu_b[b][1][:, s128], wo_t[1][:, hsl_a],
                            start=False, stop=True)
                        nc.tensor.matmul(
                            opb[:], otu_b[b][1][:, s128], wo_t[1][:, hsl_b],
                            start=False, stop=True)
                        for op, hsl in ((opa, hsl_a), (opb, hsl_b)):
                            ob = osb_pool.tile([128, 512], BF, tag="osb",
                                               name=f"ob{b}{sc16}{hsl.start}")
                            nc.vector.tensor_copy(ob[:], op[:])
                            nc.sync.dma_start(out=out[b, s128, hsl], in_=ob[:])

            def emit_oproj(b):
                for qc in range(NST):
                    emit_oproj_qc(b, qc)

            # emission order: P0 A0 P1 O0 A1 O1 — the tile scheduler fills
            # PE gaps of the ACT-bound attention with proj/o-proj matmuls.
            emit_proj(0)
            emit_attn(0)
            emit_proj(1)
            emit_oproj(0)
            emit_attn(1, inline_oproj=True)

    if not nc.is_finalized():
        nc.finalize()
    return nc


_NC = None
_RUN_KWARGS = {}
_LAST_RESULT = None


def _get_nc():
    global _NC
    if _NC is None:
        _NC = _build_nc()
    return _NC


def kernel(x, encoder_output, Wq, bq, Wk, bk, Wv, bv, Wo, bo):
    nc = _get_nc()
    xT = np.ascontiguousarray(np.asarray(x, np.float32).transpose(0, 2, 1)).astype(BF16)
    encT = np.ascontiguousarray(
        np.asarray(encoder_output, np.float32).transpose(0, 2, 1)).astype(BF16)
    Wq = np.asarray(Wq, np.float32)
    Wk = np.asarray(Wk, np.float32)
    Wv = np.asarray(Wv, np.float32)
    Wo = np.asarray(Wo, np.float32)
    in_maps = []
    for c in range(NCORES):
        csl = slice(c * CH, (c + 1) * CH)
        gsl = slice(c * D, (c + 1) * D)
        wkv_c = np.concatenate([Wk[:, gsl], Wv[:, gsl]], axis=1)
        in_maps.append({
            "xT": xT,
            "encT": encT,
            "wq": np.ascontiguousarray(Wq[:, csl]).astype(BF16),
            "wkv": np.ascontiguousarray(wkv_c).astype(BF16),
            "wo": np.ascontiguousarray(Wo[csl, :]).astype(BF16),
            "bq": np.ascontiguousarray(
                np.asarray(bq, np.float32)[csl].reshape(CH, 1)),
            "bk": np.ascontiguousarray(
                np.asarray(bk, np.float32)[gsl].reshape(D, 1)),
            "bv": np.ascontiguousarray(
                np.asarray(bv, np.float32)[gsl].reshape(D, 1)),
        })
    res = run_bass_kernel_spmd(nc, in_maps, list(range(NCORES)), **_RUN_KWARGS)
    global _LAST_RESULT
    _LAST_RESULT = res
    total = np.zeros((B, S, HID), np.float32)
    for c in range(NCORES):
        total += res.results[c]["out"].astype(np.float32)
    return total + np.asarray(bo, np.float32)
